# revision 2
# baseline (speedup 1.0000x reference)
"""Trainium2 Bass kernel for 3-layer residual LSTM decoder (B=64,T=1024,H=768).

v3: layer-pipeline across cores 0/1/2 (lags 1/3/5 slots) with col-tiled
(128x64, tile_position) matmuls throughout.  Gate columns are permuted into
six 512-chunks [i|f|o|g]x128, chunk c holding gates for h-cols 128c:128c+128.
Gates PSUM is [128, 512] per chunk-pair (c, c+3): partitions 0:64 = chunk c
(batch rows), 64:128 = chunk c+3, written by the two column tiles.  All
elementwise state math (c, h) runs on full-128-partition [128, 128] tiles
(p = b + 64*(hcol>=384)).  h^T for the next step's stationary operand comes
from two N=128 "strip" matmuls against an identity (col-tiled, no
transpose-mode, no PE mode switches).  Input-projection (bulk) GEMMs for the
next layer run col-tiled as PE filler between chunk pairs; bias is folded in
at bulk-evacuation time so pre already includes it.
"""

import numpy as np
import ml_dtypes

import concourse.bass as bass
import concourse.tile as tile
from concourse import bacc, mybir
from concourse import bass_utils

F32 = mybir.dt.float32
BF16 = mybir.dt.bfloat16
Mult = mybir.AluOpType.mult
Add = mybir.AluOpType.add

B = 64
T_FULL = 1024
IN = 512
H = 768
G = 4 * H            # 3072
OUT = 100
NCORES = 8

HC = 6               # h-col 128-tiles
NCH = 6              # gate 512-chunks
WS = 32              # steps per window
HSTEP = WS // 2
ROWS_W = B * WS      # 2048
HROWS = B * HSTEP    # 1024
MT = ROWS_W // 128   # 16 Mtiles per window
LAGS = (1, 3, 5)

Sig = mybir.ActivationFunctionType.Sigmoid
Tanh = mybir.ActivationFunctionType.Tanh

GROUPS = [[0, 1, 2, 3], [4, 5, 6, 7]]

# hT_sb index order: pair c's strips write slots 2c, 2c+1 = (kt=c, kt=3+c)
KT_ORDER = [0, 3, 1, 4, 2, 5]          # slot i holds h-col tile KT_ORDER[i]
KT_SLOT = [KT_ORDER.index(k) for k in range(6)]


def gate_perm():
    """perm[n] = original gate column at permuted column n.
    Chunk c (512 wide) = [i|f|o|g] x 128 for h-cols 128c:128c+128."""
    n = np.arange(G)
    c = n // 512
    r = n % 512
    q = r // 128
    m = r % 128
    qmap = np.array([0, 1, 3, 2])      # slot q -> original gate index
    return (qmap[q] * H + c * 128 + m).astype(np.int64)


def build_kernel(t=T_FULL, skip_ag=False):
    nwin = t // WS
    nprog = nwin + LAGS[2]

    nc = bacc.Bacc("TRN2", target_bir_lowering=False, debug=False,
                   num_devices=NCORES)

    # x shipped pre-transposed+tiled in KT_ORDER-compatible layout:
    # [kt 4 (of 6; IN=512), 128, B*t]
    xT = nc.dram_tensor("xT", [4, 128, B * t], BF16, kind="ExternalInput")
    whhT = nc.dram_tensor("whhT", [H, G], BF16, kind="ExternalInput")
    wihT = nc.dram_tensor("wihT", [H, G], BF16, kind="ExternalInput")
    biasrep = nc.dram_tensor("biasrep", [128, G], BF16, kind="ExternalInput")
    wpT = nc.dram_tensor("wpT", [H, OUT], BF16, kind="ExternalInput")
    id2 = nc.dram_tensor("id2", [128, 2, B], BF16, kind="ExternalInput")
    idf = nc.dram_tensor("idf", [128, 128], BF16, kind="ExternalInput")
    alpha = nc.dram_tensor("alpha", [128, 1], F32, kind="ExternalInput")
    hscale = nc.dram_tensor("hscale", [128, nprog], F32, kind="ExternalInput")
    out = nc.dram_tensor("out", [B * t, OUT], F32, kind="ExternalOutput")
    scratch_out = nc.dram_tensor("scratch_out", [ROWS_W, OUT], F32,
                                 kind="Internal")

    with tile.TileContext(nc) as tc:
        with (
            tc.tile_pool(name="const", bufs=1) as constp,
            tc.tile_pool(name="state", bufs=1) as statep,
            tc.tile_pool(name="pre", bufs=3) as prep,
            tc.tile_pool(name="gact", bufs=2) as gactp,
            tc.tile_pool(name="small", bufs=2) as smallp,
            tc.tile_pool(name="lhst", bufs=3) as lhstp,
            tc.tile_pool(name="bulko", bufs=2) as bulkop,
            tc.tile_pool(name="res", bufs=2) as resp,
            tc.tile_pool(name="gpsum", bufs=2, space="PSUM") as gpsump,
            tc.tile_pool(name="spsum", bufs=2, space="PSUM") as spsump,
            tc.tile_pool(name="dram", bufs=1, space="DRAM") as dramp,
        ):
            # ---------------- persistent SBUF ----------------
            # whh_sb[k-part, kt, gcol]: rows 128kt:128kt+128 of Whh^T(perm)
            whh_sb = constp.tile([128, HC, G], BF16)
            for kc in range(HC):
                nc.sync.dma_start(whh_sb[:, kc, :],
                                  whhT[128 * kc:128 * (kc + 1), :])
            wih_sb = constp.tile([128, HC, G], BF16)
            for kc in range(HC):
                nc.sync.dma_start(wih_sb[:, kc, :],
                                  wihT[128 * kc:128 * (kc + 1), :])
            bias_sb = constp.tile([128, G], BF16)
            nc.sync.dma_start(bias_sb[:], biasrep[:])
            wp_sb = constp.tile([128, HC, OUT], BF16)
            for kc in range(HC):
                nc.sync.dma_start(wp_sb[:, kc, :],
                                  wpT[128 * kc:128 * (kc + 1), :])
            # id2[:, 0, :]: rows 0:64 = I64 (even step); [:,1,:]: rows 64:128
            id_sb = constp.tile([128, 2, B], BF16)
            nc.sync.dma_start(id_sb[:], id2[:])
            idf_sb = constp.tile([128, 128], BF16)
            nc.sync.dma_start(idf_sb[:], idf[:])
            actdummy = constp.tile([1, 4], F32)
            alpha_sb = constp.tile([128, 1], F32)
            nc.sync.dma_start(alpha_sb[:], alpha[:])
            hscale_sb = constp.tile([128, nprog], F32)
            nc.sync.dma_start(hscale_sb[:], hscale[:])

            # state: [128, 384] f32; partition p = b + 64*(hcol>=384),
            # col j = hcol % 384
            c_sb = statep.tile([128, 384], F32)
            nc.vector.memset(c_sb[:], 0.0)
            # hT slots: [128, slot 6, B] bf16, slot i = h-col tile KT_ORDER[i]
            hT_sb = statep.tile([128, HC, B], BF16)
            nc.vector.memset(hT_sb[:], 0.0)
            xout_win = statep.tile([128, HC, ROWS_W], BF16)

            sp_eng = bass.OrderedSet([mybir.EngineType.SP])
            pid = nc.partition_id(engines=sp_eng)
            sel = nc.snap((pid + 7) % 8, engines=sp_eng,
                          min_val=0, max_val=7)

            # ---------------- DRAM ring ----------------
            # staging[par, half, slot, kt-slot, krow, col]:
            #   slots 0:4 = quad AllGather out (rank order), 7 = local x copy
            staging = dramp.tile([2, 2, 8, HC, 128, HROWS], BF16,
                                 name="staging")
            agx = [dramp.tile([2, HC, 128, HROWS], BF16, name=f"agx{h}")
                   for h in range(2)]
            pre_dram = dramp.tile([2, ROWS_W, G], BF16, name="pre_dram")

            # one-time zeroing (junk must be finite)
            ztile = prep.tile([128, G], BF16, tag="pre", name="ztile")
            nc.vector.memset(ztile[:], 0.0)
            for buf in (staging, pre_dram):
                v = buf[:].flatten().rearrange("(p n) -> p n", p=128)
                ncols = v.shape[1]
                off = 0
                while off < ncols:
                    w = min(G, ncols - off)
                    nc.sync.dma_start(v[:, off:off + w], ztile[:, 0:w])
                    off += w

            lh_bulk = None   # current Mtile's bulk lhs tile [128, HC, 128]
            resh_cur = None  # current half's residual window [128,HC,HROWS]
            pre_cur = None   # current Mtile's pre tile [128, G]

            def bulk_chunk(mtile, n):
                """Bulk col-tiled: pre[next-layer] chunk n (512 gate cols)
                for Mtile rows; writes ob (bf16, bias added)."""
                sp = spsump.tile([128, 512], F32, tag="sp")
                for k in range(HC):
                    lh = lh_bulk[:, k, :]
                    nc.tensor.matmul(sp[0:64, :], lh[:, 0:64],
                                     wih_sb[:, k, 512 * n:512 * (n + 1)],
                                     start=(k == 0), stop=(k == HC - 1))
                    nc.tensor.matmul(sp[64:128, :], lh[:, 64:128],
                                     wih_sb[:, k, 512 * n:512 * (n + 1)],
                                     start=(k == 0), stop=(k == HC - 1))
                ob = bulkop.tile([128, 512], BF16, tag="bob")
                nc.vector.scalar_tensor_tensor(
                    ob[:], sp[:], 1.0,
                    bias_sb[:, 512 * n:512 * (n + 1)], Mult, Add)
                nc.gpsimd.dma_start(
                    pre_dram[par_of[0], bass.ds(mtile * 128, 128),
                             512 * n:512 * (n + 1)], ob[:])

            par_of = [0]  # mutable holder for current slot parity

            def emit_pair(j, tv, tvh, cpair, pre_t, gps, E):
                """PE work for chunk-pair cpair of step (parity j)."""
                idl = id_sb[:, j, :]
                for half, ch in ((0, cpair), (1, cpair + 3)):
                    po = 64 * half
                    nc.tensor.matmul(
                        gps[po:po + 64, :], idl,
                        pre_t[:, 512 * ch:512 * (ch + 1)],
                        start=True, stop=False)
                for ki in range(HC):
                    lh = hT_sb[:, ki, :]
                    kt = KT_ORDER[ki]
                    for half, ch in ((0, cpair), (1, cpair + 3)):
                        po = 64 * half
                        nc.tensor.matmul(
                            gps[po:po + 64, :], lh,
                            whh_sb[:, kt, 512 * ch:512 * (ch + 1)],
                            start=False, stop=(ki == HC - 1))

            def emit_tail(j, tv, tvh, cpair, gps, E):
                """ACT/DVE/strip work for chunk-pair cpair after its MMs."""
                # gates chunk-pair: [i|f|o|g]x128 on both partition halves
                ga = gactp.tile([128, 512], F32, tag=f"ga{cpair}")
                nc.scalar.activation(ga[:, 0:384], gps[:, 0:384], Sig)
                nc.scalar.activation(ga[:, 384:512], gps[:, 384:512], Tanh)
                cs = c_sb[:, 128 * cpair:128 * (cpair + 1)]
                ig = smallp.tile([128, 128], F32, tag=f"ig{cpair}")
                E.tensor_mul(ig[:], ga[:, 0:128], ga[:, 384:512])
                E.tensor_mul(cs, ga[:, 128:256], cs)
                E.tensor_add(cs, cs, ig[:])
                tcb = smallp.tile([128, 128], F32, tag=f"tc{cpair}")
                nc.scalar.activation(tcb[:], cs, Tanh)
                hpair = smallp.tile([128, 128], BF16, tag=f"h{cpair}")
                E.tensor_mul(hpair[:], ga[:, 256:384], tcb[:])
                return hpair

            def emit_strip(tv, tvh, cpair, hpair, E):
                """Two col-tiled N=128 matmuls -> hT slots 2c,2c+1 + xout."""
                sp = spsump.tile([128, 512], F32, tag="sp")
                nc.tensor.matmul(sp[0:64, 0:128], hpair[:, 0:64],
                                 idf_sb[:], start=True, stop=True)
                nc.tensor.matmul(sp[64:128, 0:128], hpair[:, 64:128],
                                 idf_sb[:], start=True, stop=True)
                dst = hT_sb[:, 2 * cpair:2 * cpair + 2, :] \
                    .rearrange("p a b -> p (a b)")
                E.tensor_copy(dst, sp[:, 0:128])
                nc.vector.scalar_tensor_tensor(
                    xout_win[:, 2 * cpair:2 * cpair + 2, bass.ts(tv, B)]
                    .rearrange("p a b -> p (a b)"),
                    resh_cur[:, 2 * cpair:2 * cpair + 2, bass.ts(tvh, B)]
                    .rearrange("p a b -> p (a b)"),
                    alpha_sb[:, 0:1],
                    sp[:, 0:128], Mult, Add)

            def emit_step(iv, j, hf, mtile):
                """One LSTM step; j = step parity within Mtile."""
                tvh = iv * 2 + j
                tv = tvh + HSTEP * hf
                E = nc.vector
                pre_t = pre_cur

                gps_l = [gpsump.tile([128, 512], F32, tag=f"g{cp}")
                         for cp in range(3)]
                hpairs = [None, None, None]

                # pair0 MMs; bulk c0; pair1; tail0; strip0; bulk c1;
                # pair2; tail1; strip1; bulk c2; tail2; strip2
                emit_pair(j, tv, tvh, 0, pre_t, gps_l[0], E)
                bulk_chunk(mtile, 3 * j + 0)
                emit_pair(j, tv, tvh, 1, pre_t, gps_l[1], E)
                hpairs[0] = emit_tail(j, tv, tvh, 0, gps_l[0], E)
                emit_strip(tv, tvh, 0, hpairs[0], E)
                bulk_chunk(mtile, 3 * j + 1)
                emit_pair(j, tv, tvh, 2, pre_t, gps_l[2], E)
                hpairs[1] = emit_tail(j, tv, tvh, 1, gps_l[1], E)
                emit_strip(tv, tvh, 1, hpairs[1], E)
                bulk_chunk(mtile, 3 * j + 2)
                hpairs[2] = emit_tail(j, tv, tvh, 2, gps_l[2], E)
                emit_strip(tv, tvh, 2, hpairs[2], E)

            # ---------------- program slots ----------------
            for p in range(nprog):
                par = p % 2
                par_of[0] = par

                if p >= 1 and not skip_ag:
                    nc.gpsimd.collective_compute(
                        "AllGather", mybir.AluOpType.bypass,
                        replica_groups=GROUPS,
                        ins=[agx[1][1 - par]],
                        outs=[staging[par, 1, 0:4]])

                # scale state at window start (zero at my first real window)
                nc.vector.tensor_scalar_mul(hT_sb[:], hT_sb[:],
                                            hscale_sb[:, p:p + 1])
                nc.vector.tensor_scalar_mul(c_sb[:], c_sb[:],
                                            hscale_sb[:, p:p + 1])
                nc.scalar.activation(actdummy[:], actdummy[:], Sig)
                nc.scalar.activation(actdummy[:], actdummy[:], Tanh)

                # local x window into staging slot 7 (kt-slots for IN=512:
                # x occupies kt 0..3 -> KT_SLOT rows 0,2,4,1; zero elsewhere)
                c0 = min(p, nwin - 1) * ROWS_W
                for kx in range(4):
                    ks = KT_SLOT[kx]
                    nc.gpsimd.dma_start(
                        staging[1 - par, 0, 7, ks],
                        xT[kx, :, c0:c0 + HROWS])
                    nc.gpsimd.dma_start(
                        staging[par, 1, 7, ks],
                        xT[kx, :, c0 + HROWS:c0 + ROWS_W])

                def run_half(hf):
                    nonlocal resh_cur, lh_bulk, pre_cur
                    spar = par if hf == 0 else 1 - par
                    resh = resp.tile([128, HC, HROWS], BF16, tag="resh")
                    nc.sync.dma_start(
                        resh[:],
                        staging[spar, hf, bass.ds(sel, 1), :, :, :]
                        .transpose([2, 1, 0, 3]).squeeze(2))
                    resh_cur = resh

                    def body(iv):
                        nonlocal lh_bulk, pre_cur
                        mtile = iv if hf == 0 else iv + 8
                        lh_bulk = lhstp.tile([128, HC, 128], BF16, tag="lh")
                        bpar = (1 - par) if hf == 0 else par
                        nc.sync.dma_start(
                            lh_bulk[:],
                            staging[bpar, hf, bass.ds(sel, 1), :, :,
                                    bass.ts(iv, 128)]
                            .transpose([2, 1, 0, 3]).squeeze(2))
                        pre_t = prep.tile([128, G], BF16, tag="pre")
                        nc.sync.dma_start(
                            pre_t[:],
                            pre_dram[1 - par,
                                     bass.ds((iv * 2 + 16 * hf) * B, 128), :])
                        pre_cur = pre_t
                        emit_step(iv, 0, hf, mtile)
                        emit_step(iv, 1, hf, mtile)
                    tc.For_i_unrolled(0, 8, 1, body, 1)

                run_half(0)

                # mid-slot: ship xout h0, fire AG h0
                for kc in range(HC):
                    nc.gpsimd.dma_start(agx[0][par, kc],
                                        xout_win[:, kc, 0:HROWS])
                if p >= 1 and not skip_ag:
                    nc.gpsimd.collective_compute(
                        "AllGather", mybir.AluOpType.bypass,
                        replica_groups=GROUPS,
                        ins=[agx[0][par]],
                        outs=[staging[par, 0, 0:4]])

                run_half(1)

                # tail: ship xout h1; projection burst (full-width Mtiles)
                for kc in range(HC):
                    nc.gpsimd.dma_start(agx[1][par, kc],
                                        xout_win[:, kc, HROWS:ROWS_W])

                w2 = p - LAGS[2]
                in_range = 0 <= w2 < nwin
                for m in range(MT):
                    sp = spsump.tile([128, 512], F32, tag="sp")
                    for k in range(HC):
                        nc.tensor.matmul(
                            sp[:, 0:OUT],
                            xout_win[:, k, 128 * m:128 * (m + 1)],
                            wp_sb[:, KT_ORDER[k], :], start=(k == 0),
                            stop=(k == HC - 1))
                    po = bulkop.tile([128, OUT], F32, tag="po")
                    nc.vector.tensor_copy(po[:], sp[:, 0:OUT])
                    if in_range:
                        nc.gpsimd.dma_start(
                            out[w2 * ROWS_W + 128 * m:
                                w2 * ROWS_W + 128 * (m + 1), :], po[:])
                    else:
                        nc.gpsimd.dma_start(
                            scratch_out[128 * m:128 * (m + 1), :], po[:])

    nc.compile()
    return nc


# ---------------- host-side glue ----------------
def prep_inputs(x, Wih1, Whh1, b1, Wih2, Whh2, b2, Wih3, Whh3, b3, Wp,
                t=T_FULL):
    nwin = t // WS
    nprog = nwin + LAGS[2]
    perm = gate_perm()
    bf = ml_dtypes.bfloat16

    x = np.asarray(x, np.float32)[:, :t]
    # x tiled by h-col 128-tiles (kt 0..3 for IN=512), transposed
    xTp = np.ascontiguousarray(
        np.transpose(x, (2, 1, 0)).reshape(4, 128, t * B)).astype(bf)
    xz = np.zeros_like(xTp)

    def permT(w):
        return np.ascontiguousarray(np.asarray(w).T[:, perm]).astype(bf)

    wih1p = np.zeros((H, G), np.float32)
    wih1p[:IN] = np.asarray(Wih1).T
    wih1p = np.ascontiguousarray(wih1p[:, perm]).astype(bf)
    wihs = {0: wih1p, 1: permT(Wih2), 2: permT(Wih3)}
    whhs = {0: permT(Whh1), 1: permT(Whh2), 2: permT(Whh3)}
    biases = {0: np.asarray(b1), 1: np.asarray(b2), 2: np.asarray(b3)}
    zeroW = np.zeros((H, G), bf)
    zeroB = np.zeros((128, G), bf)

    wpT = np.ascontiguousarray(np.asarray(Wp).T).astype(bf)

    id2 = np.zeros((128, 2, B), np.float32)
    id2[0:64, 0] = np.eye(B)
    id2[64:128, 1] = np.eye(B)
    idf = np.eye(128, dtype=np.float32)

    in_maps = []
    for c in range(NCORES):
        al = np.full((128, 1), 1.0 if c in (1, 2) else 0.0, np.float32)
        hs = np.ones((128, nprog), np.float32)
        if c <= 2:
            hs[:, LAGS[c]] = 0.0
        else:
            hs[:] = 0.0
        brep = zeroB
        if c <= 2:
            brep = np.ascontiguousarray(
                np.tile(biases[c][perm][None, :], (128, 1))).astype(bf)
        in_maps.append({
            "xT": xTp if c == 0 else xz,
            "whhT": whhs.get(c, zeroW),
            "wihT": wihs.get(c, zeroW),
            "biasrep": brep,
            "wpT": wpT,
            "id2": id2.astype(bf),
            "idf": idf.astype(bf),
            "alpha": al,
            "hscale": hs,
        })
    return in_maps


_NC_CACHE = {}


def kernel(**inputs):
    if "nc" not in _NC_CACHE:
        _NC_CACHE["nc"] = build_kernel()
    nc = _NC_CACHE["nc"]
    in_maps = prep_inputs(**inputs)
    res = bass_utils.run_bass_kernel_spmd(nc, in_maps,
                                          core_ids=list(range(NCORES)))
    o = res.results[2]["out"]
    return np.ascontiguousarray(
        o.reshape(T_FULL, B, OUT).transpose(1, 0, 2)).astype(np.float32)
